# revision 1
# baseline (speedup 1.0000x reference)
"""Per-channel batched Linear (OD matrix) Trainium2 Bass kernel.

Computes out[b,o,c] = sum_t x[b,t,c] * W[c,o,t] + bias[c,o] for
x [128,48,64,64] -> [128,48,4096], W [4096,48,48], bias [4096,48].

Strategy (8 NeuronCores, channel-parallel, 512 channels/core):
  - x^T loaded HBM->SBUF with strided APs: partitions = (j2, t48) rows
    {0-47, 64-111}, free = (b, g) with 128-channel innermost runs (512B).
  - ACT casts x to bf16 with (b,g)->(g,b) permute so each channel's
    lhsT [49, 128] is contiguous (FWL-friendly); row 48/112 = ones
    (bias folded into the contraction as K=49).
  - W loaded naturally [128ch, (o,t)], cast to bf16 with o-stride 49
    (bias appended per o), PE-transposed per-o into W^T [49, 128ch]
    at row bases 0 (j0) / 64 (j1) via tile_position col packing.
  - Per-channel matmul: lhsT = x^T_aug [49,128b] (stationary, bf16),
    rhs = W^T_aug [49,48o], out psum [128b, 48o] fp32.
  - out stored naturally [b=128 partitions, (o, g)] at full DMA width.
"""

import numpy as np
import ml_dtypes

import concourse.bass as bass  # noqa: F401
import concourse.mybir as mybir
import concourse.tile as tile
from concourse import bacc
from concourse.bass_utils import run_bass_kernel_spmd

B, T, O, N = 128, 48, 48, 64
C = N * N
NCORES = 8
CS = C // NCORES  # 512 channels per core
KAUG = T + 1  # 49: contraction rows = 48 t's + 1 bias row
GH = 256  # channels per j-half
NG = CS // (2 * GH)  # 1 group of 512 channels
BC = 16  # b-chunk for x staging DMA
NBC = B // BC

F32 = mybir.dt.float32
BF16 = mybir.dt.bfloat16


def _body(tc, nc, x_d, w_d, b_d, out_d, ident_d, ones_d):
    PS = 8  # channels per psum tile (4 per j-half per bank)
    BQ = 32  # b-quarter for out tiles
    NBQ = B // BQ
    with (
        tc.tile_pool(name="const", bufs=1) as cpool,
        tc.tile_pool(name="xbf", bufs=1) as xb_pool,
        tc.tile_pool(name="wbf", bufs=4) as wb_pool,
        tc.tile_pool(name="wt", bufs=1) as wt_pool,
        tc.tile_pool(name="outs", bufs=5) as os_pool,
        tc.tile_pool(name="tpsum", bufs=3, space="PSUM") as tp_pool,
        tc.tile_pool(name="mpsum", bufs=2, space="PSUM") as mp_pool,
    ):
        idt = cpool.tile([128, 128], BF16)
        nc.sync.dma_start(idt[:, :], ident_d)

        # ---- loads (SWDGE FIFO order: W+bias, x, ones) ----
        # WT rows: {0-47: t j0, 48: bias j0, 64-111: t j1, 112: bias j1}
        # +16 pad cols so M=64 matmuls can over-read past the last channel
        wt = wt_pool.tile([128, GH * O + 16], BF16)  # col = g*O + o
        nc.vector.memset(wt[:, GH * O : GH * O + 16], 0.0)
        wbfs = {}
        for j in range(2):
            for gh in range(2):
                g0 = j * GH + gh * 128
                wbf = wb_pool.tile([128, O * T], BF16)
                nc.gpsimd.dma_start(
                    wbf[:, :], w_d[g0 : g0 + 128].rearrange("g o t -> g (o t)")
                )
                wbfs[(j, gh)] = wbf
            nc.gpsimd.dma_start(
                wt[j * 64 + T : j * 64 + T + 1, 0 : GH * O],
                b_d[j * GH : (j + 1) * GH].rearrange("g o -> (g o)").unsqueeze(0),
            )
        xbf = xb_pool.tile([128, B * GH], BF16)  # col = b*GH + g
        for bc in range(NBC):
            for j in range(2):
                src_ = x_d[
                    bc * BC : (bc + 1) * BC, :, j * GH : (j + 1) * GH
                ].rearrange("b t g -> t b g")
                dst = xbf[
                    j * 64 : j * 64 + T, bc * BC * GH : (bc + 1) * BC * GH
                ].rearrange("t (b g) -> t b g", g=GH)
                nc.gpsimd.dma_start(dst, src_)
        for j in range(2):
            nc.gpsimd.dma_start(
                xbf[j * 64 + T : j * 64 + T + 1, :], ones_d[j : j + 1, :]
            )

        # ---- W transposes into W^T ----
        for gh in range(2):
            gof = gh * 128 * O
            wt3 = wt[:, gof : gof + 128 * O].rearrange("t (g o) -> t o g", o=O)
            for oq in range(O // 4):
                pt = tp_pool.tile([128, 512], BF16)
                for os_ in range(4):
                    o = oq * 4 + os_
                    csl = slice(os_ * 128, (os_ + 1) * 128)
                    nc.tensor.transpose(
                        pt[0:T, csl], wbfs[(0, gh)][:, o * T : (o + 1) * T], idt[:, :]
                    )
                    nc.tensor.transpose(
                        pt[64 : 64 + T, csl],
                        wbfs[(1, gh)][:, o * T : (o + 1) * T],
                        idt[:, :],
                    )
                pt3 = pt[:, :].rearrange("p (o g) -> p o g", g=128)
                osl = slice(oq * 4, (oq + 1) * 4)
                if oq % 2 == 0:
                    nc.vector.tensor_copy(wt3[0:T, osl, :], pt3[0:T])
                    nc.scalar.copy(wt3[64 : 64 + T, osl, :], pt3[64 : 64 + T])
                else:
                    nc.scalar.copy(wt3[0:T, osl, :], pt3[0:T])
                    nc.vector.tensor_copy(wt3[64 : 64 + T, osl, :], pt3[64 : 64 + T])

        # ---- matmuls (out^T = W_c @ X_c^T, j-paired rows) + stores ----
        # outs tiles keyed (bq, ghalf); ghalf 0 completes at pg 15 so its
        # stores overlap the second half's matmuls.
        outs_raw = {}
        outs_tiles = {}
        xbf3 = xbf[:, :].rearrange("t (b g) -> t b g", g=GH)
        for pg in range(GH // PS):
            gh, pgh = divmod(pg, 16)
            if pgh == 0:
                for bq in range(NBQ):
                    outs = os_pool.tile([128, BQ * 128], F32)  # col = b*128+g
                    outs_raw[(bq, gh)] = outs
                    outs_tiles[(bq, gh)] = outs[:, :].rearrange(
                        "r (b p h k) -> r p h b k", p=16, h=2, k=4
                    )
            # psum col = h*512 + b*4 + kk (h = bank half, k = h*4 + kk)
            pt = mp_pool.tile([128, B * PS], F32)
            pt4 = pt[:, :].rearrange("r (h b k) -> r h b k", h=2, k=4)
            for k in range(PS):
                g = pg * PS + k
                h, kk = divmod(k, 4)
                for j in range(2):
                    r0 = j * 64
                    nc.tensor.matmul(
                        pt4[r0 : r0 + 64, h, :, kk : kk + 1],
                        lhsT=wt[r0 : r0 + KAUG, g * O : g * O + 64],
                        rhs=xbf3[r0 : r0 + KAUG, :, g : g + 1],
                        start=(kk == 0),
                        stop=(kk == 3),
                        skip_group_check=True,
                    )
            for bq in range(NBQ):
                src = pt4[:, :, bq * BQ : (bq + 1) * BQ, :]
                dst = outs_tiles[(bq, gh)][:, pgh, :, :, :]
                if (pg + bq) % 2 == 0:
                    nc.vector.tensor_copy(dst, src)
                else:
                    nc.scalar.copy(dst, src)
            if pgh == 15:
                for bq in range(NBQ):
                    for j in range(2):
                        c0 = j * GH + gh * 128
                        dst = out_d[
                            bq * BQ : (bq + 1) * BQ, :, c0 : c0 + 128
                        ].rearrange("b o g -> o b g")
                        src_ = outs_raw[(bq, gh)][j * 64 : j * 64 + O, :].rearrange(
                            "r (b g) -> r b g", g=128
                        )
                        eng = (nc.sync, nc.scalar, nc.gpsimd)[(bq * 2 + j) % 3]
                        eng.dma_start(dst, src_)


def build_program(num_devices=NCORES):
    nc = bacc.Bacc(
        "TRN2",
        target_bir_lowering=False,
        debug=False,
        enable_asserts=False,
        num_devices=num_devices,
    )
    x_d = nc.dram_tensor("x", [B, T, CS], F32, kind="ExternalInput").ap()
    w_d = nc.dram_tensor("w", [CS, O, T], F32, kind="ExternalInput").ap()
    b_d = nc.dram_tensor("bias", [CS, O], F32, kind="ExternalInput").ap()
    out_d = nc.dram_tensor("out", [B, T, CS], F32, kind="ExternalOutput").ap()
    ident_d = nc.inline_tensor(
        np.eye(128, dtype=ml_dtypes.bfloat16), name="identc"
    ).ap()
    ones_d = nc.inline_tensor(
        np.ones([2, GH * B], dtype=ml_dtypes.bfloat16), name="onesc"
    ).ap()
    with tile.TileContext(nc) as tc:
        _body(tc, nc, x_d, w_d, b_d, out_d, ident_d, ones_d)
    nc.compile()
    return nc


_CACHED_NC = None
LAST_RESULT = None


def kernel(**inputs) -> np.ndarray:
    global _CACHED_NC, LAST_RESULT
    x = np.ascontiguousarray(np.asarray(inputs["x"], dtype=np.float32)).reshape(
        B, T, C
    )
    W = np.ascontiguousarray(np.asarray(inputs["W"], dtype=np.float32))
    bias = np.ascontiguousarray(np.asarray(inputs["b"], dtype=np.float32))

    if _CACHED_NC is None:
        _CACHED_NC = build_program(NCORES)
    nc = _CACHED_NC

    in_maps = []
    for i in range(NCORES):
        sl = slice(i * CS, (i + 1) * CS)
        in_maps.append(
            {
                "x": np.ascontiguousarray(x[:, :, sl]),
                "w": np.ascontiguousarray(W[sl]),
                "bias": np.ascontiguousarray(bias[sl]),
            }
        )
    res = run_bass_kernel_spmd(nc, in_maps, core_ids=list(range(NCORES)))
    LAST_RESULT = res
    out = np.concatenate([res.results[i]["out"] for i in range(NCORES)], axis=2)
    return out.reshape(B, T, N, N)



# revision 2
# speedup vs baseline: 1.8633x; 1.8633x over previous
"""Per-channel batched Linear (OD matrix) Trainium2 Bass kernel.

Computes out[b,o,c] = sum_t x[b,t,c] * W[c,o,t] + bias[c,o] for
x [128,48,64,64] -> [128,48,4096], W [4096,48,48], bias [4096,48].

Strategy (8 NeuronCores, channel-parallel, 512 channels/core):
  Host pre-stages bf16 images so every DMA is fully contiguous and the
  PE does 256 big matmuls per core with zero on-chip transposes:
  - x image [98, 256g*128b]: rows 0-47 = t of channels 0-255 (j0),
    row 48 = ones (bias), rows 49-96 = t of channels 256-511 (j1),
    row 97 = ones. Columns g-major so pair p's 128 b-columns are the
    contiguous slice [:, p*128:(p+1)*128].
  - W^T image [98, 256p*128m]: per pair p (channels p and p+256),
    block-diagonal lhsT: rows 0-48 x cols 0-47 = aug W^T of ch p
    (48 t rows + bias row), rows 49-97 x cols 64-111 = aug W^T of
    ch p+256, zeros elsewhere (host-built).
  - Per pair: one matmul out[128m,128b] = lhsT.T @ rhs with K=98,
    M=128 (FWL-eligible), N=128, all APs contiguous.
  - psum groups of 16 pairs -> one [128,2048] f32->bf16 copy
    (DVE/ACT alternating) -> one contiguous [128,2048] bf16 store.
  - Output [128, 256p*128b] un-permuted to [B,T,N,N] f32 on host.
"""

import numpy as np
import ml_dtypes

import concourse.bass as bass  # noqa: F401
import concourse.mybir as mybir
import concourse.tile as tile
from concourse import bacc
from concourse.bass_utils import run_bass_kernel_spmd

B, T, O, N = 128, 48, 48, 64
C = N * N
NCORES = 8
CS = C // NCORES  # 512 channels per core
NPAIR = CS // 2  # 256 channel pairs per core
KR = 2 * (T + 1)  # 98 contraction rows (2 x (48 t + 1 bias))
PG = 16  # pairs per psum group (16*128 f32 cols = 8KB/part = 4 banks)
NG = NPAIR // PG  # 16 groups
NWCH = 4  # wt DMA chunks
XCOLS = NPAIR * B  # 32768

F32 = mybir.dt.float32
BF16 = mybir.dt.bfloat16
BF = ml_dtypes.bfloat16


def _body(tc, nc, x_d, w_d, out_d):
    with (
        tc.tile_pool(name="xbf", bufs=1) as xpool,
        tc.tile_pool(name="wt", bufs=1) as wpool,
        tc.tile_pool(name="outs", bufs=4) as opool,
        tc.tile_pool(name="ps", bufs=2, space="PSUM") as ppool,
    ):
        xbf = xpool.tile([128, XCOLS], BF16)
        wt = wpool.tile([128, XCOLS], BF16)
        nc.sync.dma_start(xbf[0 : KR // 2, :], x_d[0 : KR // 2, :])
        nc.scalar.dma_start(xbf[KR // 2 : KR, :], x_d[KR // 2 : KR, :])
        wc = XCOLS // NWCH
        for ch in range(NWCH):
            eng = nc.sync if ch % 2 == 0 else nc.scalar
            eng.dma_start(
                wt[0:KR, ch * wc : (ch + 1) * wc], w_d[:, ch * wc : (ch + 1) * wc]
            )

        for grp in range(NG):
            pt = ppool.tile([128, PG * B], F32)
            for k in range(PG):
                p = grp * PG + k
                nc.tensor.matmul(
                    pt[:, k * B : (k + 1) * B],
                    lhsT=wt[0:KR, p * B : (p + 1) * B],
                    rhs=xbf[0:KR, p * B : (p + 1) * B],
                    start=True,
                    stop=True,
                    skip_group_check=True,
                )
            outs = opool.tile([128, PG * B], BF16)
            if grp % 2 == 0:
                nc.vector.tensor_copy(outs[:, :], pt[:, :])
            else:
                nc.scalar.copy(outs[:, :], pt[:, :])
            nc.gpsimd.dma_start(
                out_d[:, grp * PG * B : (grp + 1) * PG * B], outs[:, :]
            )


def build_program(num_devices=NCORES):
    nc = bacc.Bacc(
        "TRN2",
        target_bir_lowering=False,
        debug=False,
        enable_asserts=False,
        num_devices=num_devices,
    )
    x_d = nc.dram_tensor("x", [KR, XCOLS], BF16, kind="ExternalInput").ap()
    w_d = nc.dram_tensor("w", [KR, XCOLS], BF16, kind="ExternalInput").ap()
    out_d = nc.dram_tensor("out", [128, XCOLS], BF16, kind="ExternalOutput").ap()
    with tile.TileContext(nc) as tc:
        _body(tc, nc, x_d, w_d, out_d)
    nc.compile()
    return nc


def _stage_inputs(x, W, bias):
    """Build per-core bf16 x/W images (host-side, not on HW critical path)."""
    xb = np.ascontiguousarray(x, dtype=np.float32).reshape(B, T, C).astype(BF)
    WTt = np.ascontiguousarray(W, dtype=np.float32).transpose(0, 2, 1).astype(BF)
    bb = np.ascontiguousarray(bias, dtype=np.float32).astype(BF)
    in_maps = []
    for i in range(NCORES):
        sl = slice(i * CS, (i + 1) * CS)
        xc = xb[:, :, sl]  # [B, T, 512]
        ximg = np.empty([KR, NPAIR, B], dtype=BF)
        ximg[0:T] = xc[:, :, 0:NPAIR].transpose(1, 2, 0)
        ximg[T] = np.ones([NPAIR, B], dtype=BF)
        ximg[T + 1 : KR - 1] = xc[:, :, NPAIR:CS].transpose(1, 2, 0)
        ximg[KR - 1] = np.ones([NPAIR, B], dtype=BF)
        wc = WTt[sl]  # [512, 48t, 48o]
        bc = bb[sl]  # [512, 48o]
        wimg = np.zeros([KR, NPAIR, B], dtype=BF)
        wimg[0:T, :, 0:O] = wc[0:NPAIR].transpose(1, 0, 2)
        wimg[T, :, 0:O] = bc[0:NPAIR]
        wimg[T + 1 : KR - 1, :, 64 : 64 + O] = wc[NPAIR:CS].transpose(1, 0, 2)
        wimg[KR - 1, :, 64 : 64 + O] = bc[NPAIR:CS]
        in_maps.append(
            {
                "x": np.ascontiguousarray(ximg.reshape(KR, XCOLS)),
                "w": np.ascontiguousarray(wimg.reshape(KR, XCOLS)),
            }
        )
    return in_maps


_CACHED_NC = None
LAST_RESULT = None


def kernel(**inputs) -> np.ndarray:
    global _CACHED_NC, LAST_RESULT
    in_maps = _stage_inputs(inputs["x"], inputs["W"], inputs["b"])

    if _CACHED_NC is None:
        _CACHED_NC = build_program(NCORES)
    nc = _CACHED_NC

    res = run_bass_kernel_spmd(nc, in_maps, core_ids=list(range(NCORES)))
    LAST_RESULT = res
    out = np.empty([B, T, C], dtype=np.float32)
    for i in range(NCORES):
        img = np.asarray(res.results[i]["out"]).reshape(128, NPAIR, B)
        sl0 = slice(i * CS, i * CS + NPAIR)
        sl1 = slice(i * CS + NPAIR, (i + 1) * CS)
        # out[b, o, p] = img[o, p, b] (ch p); img[64+o, p, b] (ch p+256)
        out[:, :, sl0] = img[0:O].transpose(2, 0, 1).astype(np.float32)
        out[:, :, sl1] = img[64 : 64 + O].transpose(2, 0, 1).astype(np.float32)
    return out.reshape(B, T, N, N)


# revision 3
# speedup vs baseline: 2.3697x; 1.2717x over previous
"""Per-channel batched Linear (OD matrix) Trainium2 Bass kernel.

Computes out[b,o,c] = sum_t x[b,t,c] * W[c,o,t] + bias[c,o] for
x [128,48,64,64] -> [128,48,4096], W [4096,48,48], bias [4096,48].

Strategy (8 NeuronCores, channel-parallel, 512 channels/core):
  Host pre-stages bf16 images so every DMA is fully contiguous and the
  PE does 256 big matmuls per core with zero on-chip transposes:
  - x image [98, 256g*128b]: rows 0-47 = t of channels 0-255 (j0),
    row 48 = ones (bias), rows 49-96 = t of channels 256-511 (j1),
    row 97 = ones. Columns g-major so pair p's 128 b-columns are the
    contiguous slice [:, p*128:(p+1)*128].
  - W^T image [98, 256p*128m]: per pair p (channels p and p+256),
    block-diagonal lhsT: rows 0-48 x cols 0-47 = aug W^T of ch p
    (48 t rows + bias row), rows 49-97 x cols 64-111 = aug W^T of
    ch p+256, zeros elsewhere (host-built).
  - Per pair: one matmul out[128m,128b] = lhsT.T @ rhs with K=98,
    M=128 (FWL-eligible), N=128, all APs contiguous.
  - psum groups of 16 pairs -> one [128,2048] f32->bf16 copy
    (DVE/ACT alternating) -> one contiguous [128,2048] bf16 store.
  - Output [128, 256p*128b] un-permuted to [B,T,N,N] f32 on host.
"""

import numpy as np
import ml_dtypes

import concourse.bass as bass  # noqa: F401
import concourse.mybir as mybir
import concourse.tile as tile
from concourse import bacc
from concourse.bass_utils import run_bass_kernel_spmd

B, T, O, N = 128, 48, 48, 64
C = N * N
NCORES = 8
CS = C // NCORES  # 512 channels per core
NPAIR = CS // 2  # 256 channel pairs per core
KR = 2 * (T + 1)  # 98 contraction rows (2 x (48 t + 1 bias))
PG = 16  # pairs per psum group (16*128 f32 cols = 8KB/part = 4 banks)
NG = NPAIR // PG  # 16 groups
NWCH = 4  # wt DMA chunks
XCOLS = NPAIR * B  # 32768

F32 = mybir.dt.float32
BF16 = mybir.dt.bfloat16
BF = ml_dtypes.bfloat16


def _body(tc, nc, x_d, w_d, out_d):
    with (
        tc.tile_pool(name="xbf", bufs=1) as xpool,
        tc.tile_pool(name="wt", bufs=1) as wpool,
        tc.tile_pool(name="outs", bufs=4) as opool,
        tc.tile_pool(name="ps", bufs=2, space="PSUM") as ppool,
    ):
        xbf = xpool.tile([128, XCOLS], BF16)
        wt = wpool.tile([128, XCOLS], BF16)
        # Interleave x/wt column-chunks on the SWDGE ring so chunk c's
        # matmuls unblock as soon as its 1.6MB lands (FIFO arrival order
        # matches need order); MMs start ~9us in and stay densely fed.
        NCH = 8
        cc = XCOLS // NCH
        for ch in range(NCH):
            sl = slice(ch * cc, (ch + 1) * cc)
            nc.gpsimd.dma_start(xbf[0:KR, sl], x_d[:, sl])
            nc.gpsimd.dma_start(wt[0:KR, sl], w_d[:, sl])

        for grp in range(NG):
            pt = ppool.tile([128, PG * B], F32)
            for k in range(PG):
                p = grp * PG + k
                nc.tensor.matmul(
                    pt[:, k * B : (k + 1) * B],
                    lhsT=wt[0:KR, p * B : (p + 1) * B],
                    rhs=xbf[0:KR, p * B : (p + 1) * B],
                    start=True,
                    stop=True,
                    skip_group_check=True,
                )
            outs = opool.tile([128, PG * B], BF16)
            if grp % 2 == 0:
                nc.vector.tensor_copy(outs[:, :], pt[:, :])
            else:
                nc.scalar.copy(outs[:, :], pt[:, :])
            seng = (nc.sync, nc.scalar, nc.gpsimd)[grp % 3]
            seng.dma_start(
                out_d[:, grp * PG * B : (grp + 1) * PG * B], outs[:, :]
            )


def build_program(num_devices=NCORES):
    nc = bacc.Bacc(
        "TRN2",
        target_bir_lowering=False,
        debug=False,
        enable_asserts=False,
        num_devices=num_devices,
    )
    x_d = nc.dram_tensor("x", [KR, XCOLS], BF16, kind="ExternalInput").ap()
    w_d = nc.dram_tensor("w", [KR, XCOLS], BF16, kind="ExternalInput").ap()
    out_d = nc.dram_tensor("out", [128, XCOLS], BF16, kind="ExternalOutput").ap()
    with tile.TileContext(nc) as tc:
        _body(tc, nc, x_d, w_d, out_d)
    nc.compile()
    return nc


def _stage_inputs(x, W, bias):
    """Build per-core bf16 x/W images (host-side, not on HW critical path)."""
    xb = np.ascontiguousarray(x, dtype=np.float32).reshape(B, T, C).astype(BF)
    WTt = np.ascontiguousarray(W, dtype=np.float32).transpose(0, 2, 1).astype(BF)
    bb = np.ascontiguousarray(bias, dtype=np.float32).astype(BF)
    in_maps = []
    for i in range(NCORES):
        sl = slice(i * CS, (i + 1) * CS)
        xc = xb[:, :, sl]  # [B, T, 512]
        ximg = np.empty([KR, NPAIR, B], dtype=BF)
        ximg[0:T] = xc[:, :, 0:NPAIR].transpose(1, 2, 0)
        ximg[T] = np.ones([NPAIR, B], dtype=BF)
        ximg[T + 1 : KR - 1] = xc[:, :, NPAIR:CS].transpose(1, 2, 0)
        ximg[KR - 1] = np.ones([NPAIR, B], dtype=BF)
        wc = WTt[sl]  # [512, 48t, 48o]
        bc = bb[sl]  # [512, 48o]
        wimg = np.zeros([KR, NPAIR, B], dtype=BF)
        wimg[0:T, :, 0:O] = wc[0:NPAIR].transpose(1, 0, 2)
        wimg[T, :, 0:O] = bc[0:NPAIR]
        wimg[T + 1 : KR - 1, :, 64 : 64 + O] = wc[NPAIR:CS].transpose(1, 0, 2)
        wimg[KR - 1, :, 64 : 64 + O] = bc[NPAIR:CS]
        in_maps.append(
            {
                "x": np.ascontiguousarray(ximg.reshape(KR, XCOLS)),
                "w": np.ascontiguousarray(wimg.reshape(KR, XCOLS)),
            }
        )
    return in_maps


_CACHED_NC = None
LAST_RESULT = None


def kernel(**inputs) -> np.ndarray:
    global _CACHED_NC, LAST_RESULT
    in_maps = _stage_inputs(inputs["x"], inputs["W"], inputs["b"])

    if _CACHED_NC is None:
        _CACHED_NC = build_program(NCORES)
    nc = _CACHED_NC

    res = run_bass_kernel_spmd(nc, in_maps, core_ids=list(range(NCORES)))
    LAST_RESULT = res
    out = np.empty([B, T, C], dtype=np.float32)
    for i in range(NCORES):
        img = np.asarray(res.results[i]["out"]).reshape(128, NPAIR, B)
        sl0 = slice(i * CS, i * CS + NPAIR)
        sl1 = slice(i * CS + NPAIR, (i + 1) * CS)
        # out[b, o, p] = img[o, p, b] (ch p); img[64+o, p, b] (ch p+256)
        out[:, :, sl0] = img[0:O].transpose(2, 0, 1).astype(np.float32)
        out[:, :, sl1] = img[64 : 64 + O].transpose(2, 0, 1).astype(np.float32)
    return out.reshape(B, T, N, N)


# revision 5
# speedup vs baseline: 2.4854x; 1.0489x over previous
"""Per-channel batched Linear (OD matrix) Trainium2 Bass kernel.

Computes out[b,o,c] = sum_t x[b,t,c] * W[c,o,t] + bias[c,o] for
x [128,48,64,64] -> [128,48,4096], W [4096,48,48], bias [4096,48].

Strategy (8 NeuronCores, channel-parallel, 512 channels/core):
  Host pre-stages bf16 images so every DMA is fully contiguous and the
  PE does 256 big matmuls per core with zero on-chip transposes:
  - x image [98, 256g*128b]: rows 0-47 = t of channels 0-255 (j0),
    row 48 = ones (bias), rows 49-96 = t of channels 256-511 (j1),
    row 97 = ones. Columns g-major so pair p's 128 b-columns are the
    contiguous slice [:, p*128:(p+1)*128].
  - W^T image [98, 256p*128m]: per pair p (channels p and p+256),
    block-diagonal lhsT: rows 0-48 x cols 0-47 = aug W^T of ch p
    (48 t rows + bias row), rows 49-97 x cols 64-111 = aug W^T of
    ch p+256, zeros elsewhere (host-built).
  - Per pair: one matmul out[128m,128b] = lhsT.T @ rhs with K=98,
    M=128 (FWL-eligible), N=128, all APs contiguous.
  - psum groups of 16 pairs -> one [128,2048] f32->bf16 copy
    (DVE/ACT alternating) -> one contiguous [128,2048] bf16 store.
  - Output [128, 256p*128b] un-permuted to [B,T,N,N] f32 on host.
"""

import numpy as np
import ml_dtypes

import concourse.bass as bass  # noqa: F401
import concourse.mybir as mybir
import concourse.tile as tile
from concourse import bacc
from concourse.bass_utils import run_bass_kernel_spmd

B, T, O, N = 128, 48, 48, 64
C = N * N
NCORES = 8
CS = C // NCORES  # 512 channels per core
NPAIR = CS // 2  # 256 channel pairs per core
KR = 2 * (T + 1)  # 98 contraction rows (2 x (48 t + 1 bias))
PG = 16  # pairs per psum group (16*128 f32 cols = 8KB/part = 4 banks)
NG = NPAIR // PG  # 16 groups
NWCH = 4  # wt DMA chunks
XCOLS = NPAIR * B  # 32768

F32 = mybir.dt.float32
BF16 = mybir.dt.bfloat16
BF = ml_dtypes.bfloat16


def _body(tc, nc, x_d, w_d, out_d):
    with (
        tc.tile_pool(name="xbf", bufs=1) as xpool,
        tc.tile_pool(name="wt", bufs=1) as wpool,
        tc.tile_pool(name="outs", bufs=4) as opool,
        tc.tile_pool(name="ps", bufs=2, space="PSUM") as ppool,
    ):
        xbf = xpool.tile([128, XCOLS], BF16)
        wt = wpool.tile([128, XCOLS], BF16)
        # Interleave x/wt column-chunks on the SWDGE ring so chunk c's
        # matmuls unblock as soon as its 1.6MB lands (FIFO arrival order
        # matches need order); MMs start ~9us in and stay densely fed.
        NCH = 8
        cc = XCOLS // NCH
        for ch in range(NCH):
            sl = slice(ch * cc, (ch + 1) * cc)
            nc.gpsimd.dma_start(xbf[0:KR, sl], x_d[:, sl])
            nc.gpsimd.dma_start(wt[0:KR, sl], w_d[:, sl])

        for grp in range(NG):
            pt = ppool.tile([128, PG * B], F32)
            for k in range(PG):
                p = grp * PG + k
                nc.tensor.matmul(
                    pt[:, k * B : (k + 1) * B],
                    lhsT=wt[0:KR, p * B : (p + 1) * B],
                    rhs=xbf[0:KR, p * B : (p + 1) * B],
                    start=True,
                    stop=True,
                    skip_group_check=True,
                )
            outs = opool.tile([96, PG * B], BF16)
            if grp % 2 == 0:
                nc.vector.tensor_copy(outs[:, :], pt[0:96, :])
            else:
                nc.scalar.copy(outs[:, :], pt[0:96, :])
            seng = nc.sync if grp % 2 == 0 else nc.scalar
            seng.dma_start(
                out_d[:, grp * PG * B : (grp + 1) * PG * B], outs[:, :]
            )


def build_program(num_devices=NCORES):
    nc = bacc.Bacc(
        "TRN2",
        target_bir_lowering=False,
        debug=False,
        enable_asserts=False,
        num_devices=num_devices,
    )
    x_d = nc.dram_tensor("x", [KR, XCOLS], BF16, kind="ExternalInput").ap()
    w_d = nc.dram_tensor("w", [KR, XCOLS], BF16, kind="ExternalInput").ap()
    out_d = nc.dram_tensor("out", [96, XCOLS], BF16, kind="ExternalOutput").ap()
    with tile.TileContext(nc) as tc:
        _body(tc, nc, x_d, w_d, out_d)
    nc.compile()
    return nc


def _stage_inputs(x, W, bias):
    """Build per-core bf16 x/W images (host-side, not on HW critical path)."""
    xb = np.ascontiguousarray(x, dtype=np.float32).reshape(B, T, C).astype(BF)
    WTt = np.ascontiguousarray(W, dtype=np.float32).transpose(0, 2, 1).astype(BF)
    bb = np.ascontiguousarray(bias, dtype=np.float32).astype(BF)
    in_maps = []
    for i in range(NCORES):
        sl = slice(i * CS, (i + 1) * CS)
        xc = xb[:, :, sl]  # [B, T, 512]
        ximg = np.empty([KR, NPAIR, B], dtype=BF)
        ximg[0:T] = xc[:, :, 0:NPAIR].transpose(1, 2, 0)
        ximg[T] = np.ones([NPAIR, B], dtype=BF)
        ximg[T + 1 : KR - 1] = xc[:, :, NPAIR:CS].transpose(1, 2, 0)
        ximg[KR - 1] = np.ones([NPAIR, B], dtype=BF)
        wc = WTt[sl]  # [512, 48t, 48o]
        bc = bb[sl]  # [512, 48o]
        wimg = np.zeros([KR, NPAIR, B], dtype=BF)
        wimg[0:T, :, 0:O] = wc[0:NPAIR].transpose(1, 0, 2)
        wimg[T, :, 0:O] = bc[0:NPAIR]
        wimg[T + 1 : KR - 1, :, O : 2 * O] = wc[NPAIR:CS].transpose(1, 0, 2)
        wimg[KR - 1, :, O : 2 * O] = bc[NPAIR:CS]
        in_maps.append(
            {
                "x": np.ascontiguousarray(ximg.reshape(KR, XCOLS)),
                "w": np.ascontiguousarray(wimg.reshape(KR, XCOLS)),
            }
        )
    return in_maps


_CACHED_NC = None
LAST_RESULT = None


def kernel(**inputs) -> np.ndarray:
    global _CACHED_NC, LAST_RESULT
    in_maps = _stage_inputs(inputs["x"], inputs["W"], inputs["b"])

    if _CACHED_NC is None:
        _CACHED_NC = build_program(NCORES)
    nc = _CACHED_NC

    res = run_bass_kernel_spmd(nc, in_maps, core_ids=list(range(NCORES)))
    LAST_RESULT = res
    out = np.empty([B, T, C], dtype=np.float32)
    for i in range(NCORES):
        img = np.asarray(res.results[i]["out"]).reshape(96, NPAIR, B)
        sl0 = slice(i * CS, i * CS + NPAIR)
        sl1 = slice(i * CS + NPAIR, (i + 1) * CS)
        # out[b, o, p] = img[o, p, b] (ch p); img[64+o, p, b] (ch p+256)
        out[:, :, sl0] = img[0:O].transpose(2, 0, 1).astype(np.float32)
        out[:, :, sl1] = img[O : 2 * O].transpose(2, 0, 1).astype(np.float32)
    return out.reshape(B, T, N, N)
